# revision 21
# baseline (speedup 1.0000x reference)
"""GCN regressor on 8 trn2 NeuronCores (raw Bass/bacc kernel).

Sharding: destination-node tiles (128 nodes) are balanced across 8 cores;
edges are partitioned by destination tile so the segment-sum stays local.
Source features are exchanged by AllGather of per-core shards, then fetched
per edge with bulk dma_gather (bf16 rows padded to 256B).

Engine roles per core, per layer:
  gpsimd: bulk dma_gather of source rows (per window x source-bucket piece)
  DVE   : one-hot chunks  oh[e, d] = norm_e * (dst_e == d)
  PE    : segT[f, d] += msg_chunk[e, f]^T @ oh[e, d]  (one PSUM bank group
          per dest tile, tile-major), then a1T @ W2 / a2T @ Wl
  ACT   : psum -> sbuf epilogues (relu, copies)

The gather stream is bucket-major within each window of tiles (so gather
calls stay few and single-bucket); the matmul stream is tile-major (so each
tile's psum accumulation group is contiguous). A static map connects them.
"""
import numpy as np
import ml_dtypes
from contextlib import ExitStack

N, E, IND, HID = 100000, 1000000, 128, 64
M = 8
P = 128
NT = 98                  # dest tiles per core
NW = 14                  # windows
WSZ = NT // NW           # 7 tiles per window
NS = NT * P              # 12544
V = M * NS               # 100352
NBUCK = 4
BSZ = V // NBUCK         # 25088 (int16-safe)
NTILE = (N + P - 1) // P # 782
FE = 128                 # padded row: 256B in bf16
RSEG = 6                 # psum bank ring for tile groups
RO = 32                  # one-hot ring (chunks)
ATR = 16                 # a1T/a2T ring (tiles)

_cache = {}


def _host_prep(x, edge_index, edge_weight):
    ei = np.asarray(edge_index).astype(np.int64)
    ew_in = np.asarray(edge_weight, dtype=np.float64)
    x = np.asarray(x, dtype=np.float32)

    src = np.concatenate([ei[0], np.arange(N, dtype=np.int64)])
    dst = np.concatenate([ei[1], np.arange(N, dtype=np.int64)])
    ew = np.concatenate([ew_in, np.ones(N, np.float64)])

    deg = np.bincount(dst, weights=ew, minlength=N)
    dinv = np.where(deg > 0, deg ** -0.5, 0.0)
    norm = (dinv[src] * ew * dinv[dst]).astype(np.float32)

    tid = dst >> 7
    cnt = np.bincount(tid, minlength=NTILE)
    order = np.argsort(-cnt, kind="stable")

    core_of_t = np.empty(NTILE, np.int64)
    rank_of_t = np.empty(NTILE, np.int64)
    for pi, t in enumerate(order):
        rnd, r = pi // M, pi % M
        core_of_t[t] = r if rnd % 2 == 0 else M - 1 - r
        rank_of_t[t] = rnd
    assign = -np.ones((M, NT), np.int64)
    assign[core_of_t, rank_of_t] = np.arange(NTILE)

    node = np.arange(N, dtype=np.int64)
    tn = node >> 7
    pos_node = core_of_t[tn] * NS + rank_of_t[tn] * P + (node & 127)

    psrc = pos_node[src]
    bucket = psrc // BSZ
    loc = psrc - bucket * BSZ

    ecore = core_of_t[tid]
    erank = rank_of_t[tid]
    ew_w = erank % NW          # window
    ew_s = erank // NW         # slot in window

    # group = (window, slot, bucket); count per (core, group)
    NG = NW * WSZ * NBUCK
    gid = ((ecore * NW + ew_w) * WSZ + ew_s) * NBUCK + bucket
    gcnt = np.bincount(gid, minlength=M * NG).reshape(M, NW, WSZ, NBUCK)
    profile = (-(-gcnt // P)).max(axis=0)          # [NW, WSZ, NBUCK] chunks

    # matmul-order group starts: order (w, s, b)
    mm_sizes = profile.reshape(-1) * P
    mm_start = np.zeros(NG + 1, np.int64)
    np.cumsum(mm_sizes, out=mm_start[1:])
    CT = int(mm_start[-1]) // P
    # gather-order group starts: order (w, b, s)
    gorder = np.transpose(profile, (0, 2, 1)).reshape(-1)  # (w, b, s)
    g_sizes = gorder * P
    g_start_go = np.zeros(NG + 1, np.int64)
    np.cumsum(g_sizes, out=g_start_go[1:])
    # map (w, s, b) -> gather-order start
    idx_wsb = (np.arange(NG).reshape(NW, WSZ, NBUCK))
    idx_wbs = np.transpose(idx_wsb, (0, 2, 1)).reshape(-1)
    g_start = np.zeros(NG + 1, np.int64)
    g_start_tmp = np.zeros(NG, np.int64)
    g_start_tmp[idx_wbs] = g_start_go[:-1]
    g_start[:-1] = g_start_tmp
    g_start[-1] = g_start_go[-1]

    sort_idx = np.argsort(gid, kind="stable")
    gs = gid[sort_idx]
    counts = np.bincount(gid, minlength=M * NG)
    starts_sorted = np.zeros(M * NG + 1, np.int64)
    np.cumsum(counts, out=starts_sorted[1:])
    within = np.arange(len(gs), dtype=np.int64) - np.repeat(
        starts_sorted[:-1], counts)
    core_s = gs // NG
    lg = gs - core_s * NG
    pos_mm = mm_start[lg] + within
    pos_g = g_start[lg] + within

    L = CT * P
    idxs = np.zeros((M, L), np.int16)
    dstl = np.zeros((M, L), np.float32)
    nrm = np.zeros((M, L), np.float32)
    idxs[core_s, pos_g] = loc[sort_idx].astype(np.int16)
    dstl[core_s, pos_mm] = (dst[sort_idx] & 127).astype(np.float32)
    nrm[core_s, pos_mm] = norm[sort_idx]

    idx_w = np.ascontiguousarray(
        np.tile(idxs.reshape(M, L // 16, 16).transpose(0, 2, 1), (1, 8, 1)))
    dst2 = dstl.reshape(M, CT, P).transpose(0, 2, 1)
    nrm2 = nrm.reshape(M, CT, P).transpose(0, 2, 1)
    iota = np.tile(np.arange(P, dtype=np.float32), (P, 1))
    cst = np.ascontiguousarray(np.concatenate(
        [np.tile(iota[None], (M, 1, 1)), dst2, nrm2],
        axis=2)).astype(np.float32)

    xpad = np.vstack([x, np.zeros((NTILE * P - N + P, IND), np.float32)])
    ntab = np.where(assign >= 0, assign, NTILE)[:, :, None] * P + np.arange(P)
    xt = xpad[ntab.reshape(M, NS)]
    xt = np.ascontiguousarray(xt.transpose(0, 2, 1)).astype(ml_dtypes.bfloat16)

    return dict(idx_w=idx_w, cst=cst, xt=xt, assign=assign,
                profile=profile, CT=CT)


def _build_nc(profile, CT):
    import concourse.bacc as bacc
    import concourse.mybir as mybir
    from concourse.library_config import mlp

    # gather pieces: (w, b) -> chunks, and gather-order window starts
    piece_wb = profile.sum(axis=1)                     # [NW, NBUCK]
    win_chunks = piece_wb.sum(axis=1)                  # [NW]
    win_start = np.zeros(NW + 1, np.int64)
    np.cumsum(win_chunks, out=win_start[1:])
    MAXW = int(win_chunks.max())
    # gather-order offset of (w, b) piece within its window
    pboff = np.zeros((NW, NBUCK), np.int64)
    for w in range(NW):
        o = 0
        for b in range(NBUCK):
            pboff[w, b] = o
            o += int(piece_wb[w, b])
    # gather-order offset of (w, s, b) group within window
    gwoff = np.zeros((NW, WSZ, NBUCK), np.int64)
    for w in range(NW):
        for b in range(NBUCK):
            o = pboff[w, b]
            for s in range(WSZ):
                gwoff[w, s, b] = o
                o += int(profile[w, s, b])

    # matmul-order schedule: chunk -> (w, s, b, c)
    sched = []
    for w in range(NW):
        for s in range(WSZ):
            for b in range(NBUCK):
                for c in range(int(profile[w, s, b])):
                    sched.append((w, s, b, c))
    NCH = len(sched)
    assert NCH == CT
    first_chunk, last_chunk = {}, {}
    for cc, (w, s, b, c) in enumerate(sched):
        first_chunk.setdefault((w, s), cc)
        last_chunk[(w, s)] = cc
    for w in range(NW):
        for s in range(WSZ):
            assert (w, s) in first_chunk, "empty tile"
    # matmul-order chunk count through window w (for gather WAR)
    mm_win_end = np.zeros(NW + 1, np.int64)
    for cc, (w, s, b, c) in enumerate(sched):
        mm_win_end[w + 1] = cc + 1

    nc = bacc.Bacc("TRN2")
    xtd = nc.dram_tensor("xt", [P, NS], mybir.dt.bfloat16, kind="ExternalInput")
    idxd = nc.dram_tensor("idx_w", [P, CT * 8], mybir.dt.int16,
                          kind="ExternalInput")
    cstd = nc.dram_tensor("cst", [P, P + 2 * CT], mybir.dt.float32,
                          kind="ExternalInput")
    w1d = nc.dram_tensor("w1", [IND, HID], mybir.dt.bfloat16,
                         kind="ExternalInput")
    w2d = nc.dram_tensor("w2", [HID, HID], mybir.dt.bfloat16,
                         kind="ExternalInput")
    wld = nc.dram_tensor("wl", [HID, 1], mybir.dt.bfloat16,
                         kind="ExternalInput")
    outd = nc.dram_tensor("outd", [P, NT], mybir.dt.float32,
                          kind="ExternalOutput")

    g1_shard = nc.dram_tensor("g1_shard", [NS, FE], mybir.dt.bfloat16)
    g2_shard = nc.dram_tensor("g2_shard", [NS, FE], mybir.dt.bfloat16)
    g1_full = nc.dram_tensor("g1_full", [V, FE], mybir.dt.bfloat16,
                             addr_space="Shared")
    g2_full = nc.dram_tensor("g2_full", [V, FE], mybir.dt.bfloat16,
                             addr_space="Shared")
    tables = (g1_full, g2_full)

    with ExitStack() as ctx:
        sb = lambda nm, sh, dt: ctx.enter_context(nc.sbuf_tensor(nm, sh, dt))
        sem = lambda nm: ctx.enter_context(nc.semaphore(nm))

        xt_t = sb("xt_t", [P, NS], mybir.dt.bfloat16)
        idx_t = sb("idx_t", [P, CT * 8], mybir.dt.int16)
        cst = sb("cst_t", [P, P + 2 * CT], mybir.dt.float32)
        w1_t = sb("w1_t", [IND, HID], mybir.dt.bfloat16)
        w2_t = sb("w2_t", [HID, HID], mybir.dt.bfloat16)
        wl_t = sb("wl_t", [HID, 1], mybir.dt.bfloat16)
        msg_ring = sb("msg_ring", [P, 2, MAXW, FE], mybir.dt.bfloat16)
        oh_ring = sb("oh_ring", [P, RO, P], mybir.dt.bfloat16)
        # g2_buf aliases xt (xt is dead once phase A matmuls finish; the
        # first g2 write is sem-ordered after them via mm2 -> relu -> seg)
        gbuf = [sb("g1_buf", [P, NT, FE], mybir.dt.bfloat16),
                xt_t[:].rearrange("p (i f) -> p i f", f=FE)]
        at_ring = sb("at_ring", [HID, ATR, P], mybir.dt.bfloat16)
        out_buf = sb("out_buf", [P, NT], mybir.dt.float32)

        # one full 2KB bank per concurrent accumulation slot: matmul start
        # zeroes the whole bank-granular zero region on HW
        seg_ps = ctx.enter_context(
            nc.psum_tensor("seg_ps", [HID, RSEG, 512], mybir.dt.float32))
        misc_ps = ctx.enter_context(
            nc.psum_tensor("misc_ps", [P, 2, 512], mybir.dt.float32))
        ga_ps = [misc_ps[:, 0, 0:HID], misc_ps[:, 1, 0:HID]]

        s_in = sem("s_in")
        s_mma = sem("s_mma")
        s_g1c = sem("s_g1c")
        s_shard = [sem("s_shard0"), sem("s_shard1")]
        s_cc = sem("s_cc")
        # per (slot parity, bucket) gather sems, per layer
        s_msgs = [[sem(f"s_msg{l}_{i}") for i in range(2 * NBUCK)]
                  for l in (0, 1)]
        s_oh = [sem("s_oh0"), sem("s_oh1")]
        s_mm = [sem("s_mm0"), sem("s_mm1")]
        s_ep = [sem("s_ep0"), sem("s_ep1")]
        s_mm2 = [sem("s_mm20"), sem("s_mm21")]
        s_epc = [sem("s_epc0"), sem("s_epc1")]
        s_fin = sem("s_fin")
        s_z = sem("s_z")

        with nc.Block() as block:

            @block.sync
            def _(sync):
                sync.dma_start(idx_t[:], idxd[:]).then_inc(s_in, 16)
                sync.dma_start(cst[:], cstd[:]).then_inc(s_in, 16)
                sync.dma_start(xt_t[:], xtd[:]).then_inc(s_in, 16)
                sync.dma_start(w1_t[:], w1d[:]).then_inc(s_in, 16)
                sync.dma_start(w2_t[:], w2d[:]).then_inc(s_in, 16)
                sync.dma_start(wl_t[:], wld[:]).then_inc(s_in, 16)
                sync.wait_ge(s_z, 1)
                sync.wait_ge(s_g1c, NT)
                sync.dma_start(
                    g1_shard[:].rearrange("(i p) f -> p i f", p=P),
                    gbuf[0][:]).then_inc(s_shard[0], 16)
                sync.wait_ge(s_z, 2)
                sync.wait_ge(s_epc[0], NT)
                sync.dma_start(
                    g2_shard[:].rearrange("(i p) f -> p i f", p=P),
                    gbuf[1][:]).then_inc(s_shard[1], 16)
                sync.wait_ge(s_epc[1], NT)
                sync.dma_start(outd[:], out_buf[:]).then_inc(s_fin, 16)
                sync.wait_ge(s_fin, 16)

            @block.gpsimd
            def _(gpsimd):
                gpsimd.load_library(mlp)
                gpsimd.wait_ge(s_in, 96)
                gpsimd.wait_ge(s_shard[0], 16)
                gpsimd.collective_compute(
                    "AllGather", mybir.AluOpType.bypass,
                    ins=[g1_shard[:]], outs=[g1_full[:]],
                    replica_groups=[list(range(M))],
                ).then_inc(s_cc, 1)
                GMAX = 8   # max chunks per dma_gather call (1024 idxs: HW limit)
                for l in (0, 1):
                    gpsimd.wait_ge(s_cc, l + 1)
                    for w in range(NW):
                        if w >= 2:
                            gpsimd.wait_ge(s_mm[l], int(mm_win_end[w - 1]))
                        for b in range(NBUCK):
                            pc = int(piece_wb[w, b])
                            if pc == 0:
                                continue
                            for q0 in range(0, pc, GMAX):
                                qc = min(GMAX, pc - q0)
                                lo = int(win_start[w] + pboff[w, b]) + q0
                                n = qc * P
                                gpsimd.dma_gather(
                                    msg_ring[:, w % 2,
                                             int(pboff[w, b]) + q0:
                                             int(pboff[w, b]) + q0 + qc, :],
                                    tables[l][b * BSZ:(b + 1) * BSZ, :],
                                    idx_t[:, lo * 8:lo * 8 + n // 16],
                                    n, n, FE,
                                ).then_inc(s_msgs[l][(w % 2) * NBUCK + b], 16)
                    if l == 0:
                        gpsimd.wait_ge(s_shard[1], 16)
                        gpsimd.collective_compute(
                            "AllGather", mybir.AluOpType.bypass,
                            ins=[g2_shard[:]], outs=[g2_full[:]],
                            replica_groups=[list(range(M))],
                        ).then_inc(s_cc, 1)

            @block.vector
            def _(vector):
                vector.memset(gbuf[0][:, :, HID:FE], 0).then_inc(s_z, 1)
                vector.wait_ge(s_in, 96)
                # g2_buf aliases xt: zero its pad columns only after phase A
                vector.wait_ge(s_mma, NT)
                vector.memset(gbuf[1][:, :, HID:FE], 0).then_inc(s_z, 1)
                for l in (0, 1):
                    if l == 1:
                        # oh ring handoff between layers
                        vector.wait_ge(s_mm[0], NCH)
                    ndone = 0
                    for cc in range(NCH):
                        if cc % 16 == 0 and cc + 16 > RO:
                            vector.wait_ge(s_mm[l], min(cc + 16, NCH) - RO)
                        ts = vector.tensor_scalar(
                            out=oh_ring[:, cc % RO, :], in0=cst[:, 0:P],
                            scalar1=cst[:, P + cc:P + cc + 1],
                            scalar2=cst[:, P + CT + cc:P + CT + cc + 1],
                            op0=mybir.AluOpType.is_equal,
                            op1=mybir.AluOpType.mult,
                        )
                        if cc % 4 == 3 or cc == NCH - 1:
                            ts.then_inc(s_oh[l], cc + 1 - ndone)
                            ndone = cc + 1

            @block.scalar
            def _(scalar):
                Relu = mybir.ActivationFunctionType.Relu
                Copy = mybir.ActivationFunctionType.Copy
                scalar.wait_ge(s_in, 96)
                for i in range(NT):
                    scalar.wait_ge(s_mma, i + 1)
                    scalar.activation(
                        out=gbuf[0][:, i, 0:HID], in_=ga_ps[i % 2], func=Copy,
                    ).then_inc(s_g1c, 1)

                def relus(l, w):
                    for s in range(WSZ):
                        g = w * WSZ + s
                        scalar.wait_ge(s_mm[l], last_chunk[(w, s)] + 1)
                        if g >= ATR:
                            scalar.wait_ge(s_mm2[l], g - ATR + 1)
                        scalar.activation(
                            out=at_ring[:, g % ATR, :],
                            in_=seg_ps[:, g % RSEG, 0:P],
                            func=Relu,
                        ).then_inc(s_ep[l], 1)

                def copies(l, w):
                    for s in range(WSZ):
                        g = w * WSZ + s
                        i = w + NW * s
                        scalar.wait_ge(s_mm2[l], g + 1)
                        if l == 0:
                            scalar.activation(
                                out=gbuf[1][:, i, 0:HID], in_=ga_ps[g % 2],
                                func=Copy,
                            ).then_inc(s_epc[0], 1)
                        else:
                            scalar.activation(
                                out=out_buf[:, i:i + 1],
                                in_=ga_ps[g % 2][:, 0:1], func=Copy,
                            ).then_inc(s_epc[1], 1)

                for l in (0, 1):
                    for w in range(NW):
                        relus(l, w)
                        if w >= 1:
                            copies(l, w - 1)
                    copies(l, NW - 1)

            @block.tensor
            def _(tensor):
                tensor.wait_ge(s_in, 96)
                for i in range(NT):
                    if i >= 2:
                        tensor.wait_ge(s_g1c, i - 1)
                    tensor.matmul(
                        out=ga_ps[i % 2],
                        lhsT=xt_t[:, i * P:(i + 1) * P],
                        rhs=w1_t[:, :],
                        start=True, stop=True,
                    ).then_inc(s_mma, 1)

                def mm2s(l, w):
                    for s in range(WSZ):
                        g = w * WSZ + s
                        tensor.wait_ge(s_ep[l], g + 1)
                        if g >= 2:
                            tensor.wait_ge(s_epc[l], g - 1)
                        tensor.matmul(
                            out=ga_ps[g % 2] if l == 0 else ga_ps[g % 2][:, 0:1],
                            lhsT=at_ring[:, g % ATR, :],
                            rhs=w2_t[:, :] if l == 0 else wl_t[:, :],
                            start=True, stop=True,
                        ).then_inc(s_mm2[l], 1)

                GMAX = 8
                for l in (0, 1):
                    ncall = [0] * (2 * NBUCK)
                    waited = set()
                    for cc, (w, s, b, c) in enumerate(sched):
                        g = w * WSZ + s
                        if (w, b) not in waited:
                            waited.add((w, b))
                            pc = int(piece_wb[w, b])
                            if pc > 0:
                                sl = (w % 2) * NBUCK + b
                                ncall[sl] += (pc + GMAX - 1) // GMAX
                                tensor.wait_ge(s_msgs[l][sl], ncall[sl] * 16)
                        if cc % 4 == 0:
                            tensor.wait_ge(s_oh[l], min(cc + 4, NCH))
                        st = first_chunk[(w, s)] == cc
                        if st and g >= RSEG:
                            tensor.wait_ge(s_ep[l], g - RSEG + 1)
                        moff = int(gwoff[w, s, b]) + c
                        tensor.matmul(
                            out=seg_ps[:, g % RSEG, 0:P],
                            lhsT=msg_ring[:, w % 2, moff, 0:HID],
                            rhs=oh_ring[:, cc % RO, :],
                            start=st, stop=last_chunk[(w, s)] == cc,
                        ).then_inc(s_mm[l], 1)
                        if cc == int(mm_win_end[w + 1]) - 1 and w >= 1:
                            mm2s(l, w - 1)
                    mm2s(l, NW - 1)

    nc.compile()
    return nc


def _run(nc, in_maps):
    from concourse.bass_utils import run_bass_kernel_spmd
    return run_bass_kernel_spmd(nc, in_maps, core_ids=list(range(M)))


def kernel(x, edge_index, edge_weight, W1, b1, W2, b2, Wl, bl):
    W1 = np.asarray(W1, np.float32)
    b1 = np.asarray(b1, np.float32)
    W2 = np.asarray(W2, np.float32)
    b2 = np.asarray(b2, np.float32)
    Wl = np.asarray(Wl, np.float32)
    bl = np.asarray(bl, np.float32)
    if np.any(b1 != 0) or np.any(b2 != 0):
        return _kernel_numpy(x, edge_index, edge_weight, W1, b1, W2, b2, Wl, bl)

    prep = _host_prep(x, edge_index, edge_weight)
    in_maps = _in_maps(prep, W1, W2, Wl)
    nc = _get_nc(prep)
    res = _run(nc, in_maps)
    return _assemble(res.results, prep, bl)


def _get_nc(prep):
    key = (prep["CT"], prep["profile"].tobytes())
    if key not in _cache:
        _cache.clear()
        _cache[key] = _build_nc(prep["profile"], prep["CT"])
    return _cache[key]


def _in_maps(prep, W1, W2, Wl):
    w1b = np.ascontiguousarray(W1).astype(ml_dtypes.bfloat16)
    w2b = np.ascontiguousarray(W2).astype(ml_dtypes.bfloat16)
    wlb = np.ascontiguousarray(Wl).astype(ml_dtypes.bfloat16)
    return [{
        "xt": prep["xt"][c], "idx_w": prep["idx_w"][c], "cst": prep["cst"][c],
        "w1": w1b, "w2": w2b, "wl": wlb,
    } for c in range(M)]


def _assemble(results, prep, bl):
    out_full = np.zeros(NTILE * P + P, np.float32)
    assign = prep["assign"]
    for c in range(M):
        o = np.asarray(results[c]["outd"])
        for i in range(NT):
            t = assign[c, i]
            if t >= 0:
                out_full[t * P:(t + 1) * P] = o[:, i]
    return (out_full[:N] + np.float32(np.asarray(bl).reshape(-1)[0])).astype(
        np.float32)


def _kernel_numpy(x, edge_index, edge_weight, W1, b1, W2, b2, Wl, bl):
    x = np.asarray(x, dtype=np.float32)
    ei = np.asarray(edge_index).astype(np.int64)
    ew_in = np.asarray(edge_weight, dtype=np.float32)
    loop = np.arange(N, dtype=np.int64)
    src = np.concatenate([ei[0], loop])
    dst = np.concatenate([ei[1], loop])
    ew = np.concatenate([ew_in, np.ones(N, dtype=np.float32)])
    deg = np.bincount(dst, weights=ew, minlength=N)
    dinv = np.where(deg > 0, 1.0 / np.sqrt(deg), 0.0).astype(np.float32)
    norm = (dinv[src] * ew * dinv[dst]).astype(np.float32)

    def prop(h):
        msg = h[src] * norm[:, None]
        out = np.zeros((N, h.shape[1]), np.float32)
        np.add.at(out, dst, msg)
        return out

    h = np.maximum(prop(x @ W1) + b1, 0.0)
    h = np.maximum(prop(h @ W2) + b2, 0.0)
    return (h @ Wl + bl).squeeze(-1).astype(np.float32)
